# revision 18
# baseline (speedup 1.0000x reference)
"""Grouped-Query Attention (S=2048, NQ=32, NKV=8, D=128, HID=4096) on 8 TRN2 NeuronCores.

Sharding: tensor-parallel over heads. Core c owns KV head c and its G=4
query heads (rows c*512..(c+1)*512 of Wq, c*128..(c+1)*128 of Wk/Wv, and
columns c*512..(c+1)*512 of Wo).  Each core computes a partial output
(row-parallel Wo); the host sums the 8 partials.

All matmuls run in fp16 (1 cycle/row on PE, better mantissa than bf16)
with fp32 PSUM accumulation.  Schedule:
  - stage A: projections, streaming x once; produces qT/kT/vT [d, s]
    (d on partitions) plus v[j, d] via 16 PE transposes.
  - stage B: scores computed transposed S^T[j,i] = kT-slice.T @ qT,
    exp on Act (scale folded in, bias -1 for fp16 range), rowsums
    accumulated on DVE in fp16 (2x mode), cross-partition sum via a
    ones-matmul, 1/r as exp(-ln(r)) on Act (ln+exp live in the same
    activation table, so no table reloads).
  - stage C: output projection, software-pipelined INTO stage B: one
    [128,512] output tile (4 accumulating matmuls) is issued after every
    odd ctx matmul, so the PE stays busy while Act drains the exps.
    C(t-1) runs inside B(t); per t the 32 output tiles exactly fill the
    4*8 interleave slots.
  - the 4MB wo DMA is issued after stage A's first chunk so it does not
    block the x stream / small weights in the single DMA queue (this was
    a 30us PE stall at startup).
"""

import os
import sys

import numpy as np

for _p in ("/opt/trn_rl_repo", "/root/.axon_site/_ro/trn_rl_repo"):
    if os.path.isdir(_p) and _p not in sys.path:
        sys.path.insert(0, _p)

import concourse.bass as bass
import concourse.bacc as bacc
import concourse.mybir as mybir
import concourse.tile as tile
from concourse.bass_utils import run_bass_kernel_spmd
from concourse.masks import make_identity

P = 128          # partitions / head dim / PE tile
S = 2048         # sequence length
HID = 4096       # hidden dim
NCORES = 8
NH = 4           # q heads per core
DQ = NH * P      # per-core q width (512)
SC = 512         # free-dim chunk (PSUM bank = 512 fp32)
NKT = HID // P   # 32 contraction tiles over hidden
NCH = S // SC    # 4 sequence chunks
NJT = S // P     # 16 key tiles
NOC = HID // SC  # 8 out column chunks
SCALE = float(P) ** -0.5
F16 = mybir.dt.float16
F32 = mybir.dt.float32
F16NP = np.float16

_CACHE = {}


def _build():
    nc = bacc.Bacc(None, target_bir_lowering=False)
    xT = nc.declare_dram_parameter("xT", [HID, S], F16, isOutput=False)
    WqT = nc.declare_dram_parameter("WqT", [HID, DQ], F16, isOutput=False)
    WkT = nc.declare_dram_parameter("WkT", [HID, P], F16, isOutput=False)
    WvT = nc.declare_dram_parameter("WvT", [HID, P], F16, isOutput=False)
    bvp = nc.declare_dram_parameter("bvp", [P, 1], F32, isOutput=False)
    WoT = nc.declare_dram_parameter("WoT", [DQ, HID], F16, isOutput=False)
    onesd = nc.declare_dram_parameter("onesd", [P, P], F16, isOutput=False)
    out = nc.declare_dram_parameter("out", [S, HID], F16, isOutput=True)

    EXP = mybir.ActivationFunctionType.Exp
    LN = mybir.ActivationFunctionType.Ln
    IDENT = mybir.ActivationFunctionType.Identity

    with tile.TileContext(nc) as tc:
        with (
            tc.tile_pool(name="consts", bufs=1) as consts,
            tc.tile_pool(name="acts", bufs=1) as acts,
            tc.tile_pool(name="xin", bufs=16) as xin,
            tc.tile_pool(name="epool", bufs=5) as epool,
            tc.tile_pool(name="rpool", bufs=2) as rpool,
            tc.tile_pool(name="opool", bufs=8) as opool,
        ):
            # ---- constants first (warmup matmuls depend only on these).
            # Queue order matters: everything the first stage-A matmuls
            # need (wk, wv, wq) goes first; wo is DMA'd later.
            ident = consts.tile([P, P], F16)
            make_identity(nc, ident)
            # Preload the one activation table that serves Exp, Ln AND
            # Identity (act_info.json act_func_sets[6],
            # "natural_log_exp_and_others").  Without this the table-load
            # pass greedily ping-pongs between the exp-only and ln-only
            # tables: 33 ACT_TABLE_LOADs (~42us) on the Act critical path.
            _tl = mybir.InstLoadActFuncSet(
                name=nc.get_next_instruction_name(), ins=[], outs=[])
            _tl.act_func_set_id = 6
            nc.scalar.add_instruction(_tl)
            nbias = consts.tile([P, 1], F32)
            nc.gpsimd.memset(nbias, -1.0)
            # All weights go through the Act engine's DMA queue so the x
            # stream (sync queue) is never stuck behind them; the two HWDGE
            # queues drain in parallel.  Split wk/wv/wq into kt-octet parts
            # interleaved in deadline order, so the first k/v/q matmuls can
            # start after ~0.5MB instead of after 6MB.
            wk = consts.tile([P, NKT, P], F16)
            wv = consts.tile([P, NKT, P], F16)
            wq = consts.tile([P, NKT, DQ], F16)
            wk_src = WkT[:, :].rearrange("(g kt p) d -> p g kt d", p=P, g=4)
            wv_src = WvT[:, :].rearrange("(g kt p) d -> p g kt d", p=P, g=4)
            wq_src = WqT[:, :].rearrange("(g kt p) d -> p g kt d", p=P, g=4)
            for g in range(4):
                sl = slice(g * 8, (g + 1) * 8)
                nc.scalar.dma_start(out=wk[:, sl, :], in_=wk_src[:, g])
                nc.scalar.dma_start(out=wv[:, sl, :], in_=wv_src[:, g])
                nc.scalar.dma_start(out=wq[:, sl, :], in_=wq_src[:, g])
            bv_sb = consts.tile([P, 1], F32)
            nc.scalar.dma_start(out=bv_sb, in_=bvp[:, :])
            onesf = consts.tile([P, P], F16)
            nc.scalar.dma_start(out=onesf, in_=onesd[:, :])
            wo = consts.tile([P, NH, HID], F16)  # DMA'd mid-x-stream below

            # ---- persistent activations (fp16) ----
            qT = acts.tile([P, NH, S], F16)     # per head: [128 d, 2048 s]
            kT = acts.tile([P, S], F16)         # [128 d, 2048 s]
            vT = acts.tile([P, S], F16)         # [128 d, 2048 s]
            v = acts.tile([P, NJT, P], F16)     # [128 j, jt, 128 d]
            ctxT = acts.tile([P, NH, S], F16)   # per head: [128 d, 2048 i]

            # ---- PE warmup: keep TensorE busy during initial weight DMAs so
            # the HAM clock-gate is released before real matmuls start ----
            with tc.tile_pool(name="pwarm", bufs=1, space="PSUM") as pwarm:
                wt = pwarm.tile([P, P], F16, name="warm")
                for _ in range(36):
                    nc.tensor.transpose(wt, ident, ident)

            # ---- stage A: projections (stream x once).  Chunk 0 lags the
            # q matmuls by LAG k-tiles so the k/v matmuls cover the wq DMA.
            # Chunk 3 skips q entirely: its q-projection is interleaved into
            # B(t=0) below, where it plays the role the output projection
            # plays in later B chunks (PE filler while Act drains exps). ----
            LAG = 4
            with tc.tile_pool(name="pacc", bufs=1, space="PSUM") as pacc:
                # Issue ALL x-tile DMAs up front (textual order = sync-queue
                # order).  The 16-buffer ring auto-throttles: tile i's DMA
                # fires once tile i-16's consumers finish, so each chunk's
                # tiles prefetch during the previous chunk's compute.  This
                # keeps the k/v-only stretches (chunk-0 lag phase, chunk 3)
                # from starving on DMA rate.
                all_x = {}
                for c in range(NCH):
                    if c == NCH - 1:
                        # wo rides the ring-gated sync queue here: it lands
                        # during chunk 3's compute (~100us), far ahead of its
                        # ~215us deadline, and keeps the 4MB transfer out of
                        # the bandwidth-critical first 60us.
                        nc.sync.dma_start(
                            out=wo,
                            in_=WoT[:, :].rearrange("(dt p) o -> p dt o", p=P))
                    for kt in range(NKT):
                        xt = all_x[(c, kt)] = xin.tile([P, SC], F16, name="xt")
                        nc.sync.dma_start(
                            out=xt,
                            in_=xT[kt * P:(kt + 1) * P, c * SC:(c + 1) * SC])
                for c in range(NCH):
                    s0 = c * SC
                    do_q = c != NCH - 1
                    lag = LAG if c == 0 else 0
                    if do_q:
                        q_ps = [pacc.tile([P, SC], F32, tag="pq%d" % m,
                                          name="q_ps%d" % m) for m in range(NH)]
                    k_ps = pacc.tile([P, SC], F32, tag="pk")
                    v_ps = pacc.tile([P, SC], F32, tag="pv")
                    for kt in range(NKT + lag):
                        if kt < NKT:
                            xt = all_x[(c, kt)]
                            st, sp = kt == 0, kt == NKT - 1
                            nc.tensor.matmul(k_ps, lhsT=wk[:, kt, :], rhs=xt,
                                             start=st, stop=sp)
                            nc.tensor.matmul(v_ps, lhsT=wv[:, kt, :], rhs=xt,
                                             start=st, stop=sp)
                        qk = kt - lag
                        if do_q and 0 <= qk < NKT:
                            xq = all_x[(c, qk)]
                            for m in range(NH):
                                nc.tensor.matmul(q_ps[m],
                                                 lhsT=wq[:, qk, m * P:(m + 1) * P],
                                                 rhs=xq, start=qk == 0,
                                                 stop=qk == NKT - 1)
                    # PSUM copy-out, split DVE/Act and ordered so the next
                    # chunk's matmuls (k, v, q0..q3) find their banks free
                    # as they need them.
                    nc.vector.tensor_copy(out=kT[:, s0:s0 + SC], in_=k_ps)
                    # v = x @ Wv.T + bv  (bias is per-partition in [d, s] layout)
                    nc.scalar.activation(out=vT[:, s0:s0 + SC], in_=v_ps,
                                         func=IDENT, bias=bv_sb, scale=1.0)
                    if do_q:
                        nc.vector.tensor_copy(out=qT[:, 0, s0:s0 + SC], in_=q_ps[0])
                        nc.vector.tensor_copy(out=qT[:, 1, s0:s0 + SC], in_=q_ps[1])
                        nc.scalar.activation(out=qT[:, 2, s0:s0 + SC], in_=q_ps[2],
                                             func=IDENT, scale=1.0)
                        nc.scalar.activation(out=qT[:, 3, s0:s0 + SC], in_=q_ps[3],
                                             func=IDENT, scale=1.0)
                    # v[j, d] via PE transpose, interleaved per chunk
                    for jj in range(SC // P):
                        jt = c * (SC // P) + jj
                        t_ps = pacc.tile([P, P], F16, tag="ptr", bufs=2)
                        nc.tensor.transpose(t_ps, vT[:, jt * P:(jt + 1) * P], ident)
                        nc.vector.tensor_copy(out=v[:, jt, :], in_=t_ps)

            # ---- stages B+C: attention with PE filler work software-
            # pipelined into the key-tile loop.  Per (t,h) slot the jt loop
            # yields 8 interleave positions; fillers are closures popped
            # from `work`: at t=0 the deferred chunk-3 q-projection (32
            # positions), at t>0 the previous chunk's output projection
            # (32 tiles).  PSUM: pscore 2x[128,1024] + pctx 2 + pout 2 =
            # exactly 8 banks. ----
            s3 = (NCH - 1) * SC
            with tc.tile_pool(name="pbc", bufs=1, space="PSUM") as pbc:
                work = []

                def c_group(mt, oc, cp_eng=0):
                    m0, o0 = mt * P, oc * SC
                    o_ps = pbc.tile([P, SC], F32, tag="pout", bufs=2, name="o_ps")
                    for dt_ in range(NH):
                        nc.tensor.matmul(o_ps, lhsT=ctxT[:, dt_, m0:m0 + P],
                                         rhs=wo[:, dt_, o0:o0 + SC],
                                         start=dt_ == 0, stop=dt_ == NH - 1)
                    ob = opool.tile([P, SC], F16)
                    if cp_eng == 0:
                        nc.vector.tensor_copy(out=ob, in_=o_ps)
                    else:
                        nc.scalar.activation(out=ob, in_=o_ps, func=IDENT,
                                             scale=1.0)
                    nc.sync.dma_start(out=out[m0:m0 + P, o0:o0 + SC], in_=ob)

                # Deferred chunk-3 q-projection: two passes (head pairs
                # (0,1) then (2,3)), each re-streaming x chunk 3; position g
                # covers k-tiles 2g,2g+1 for both heads of the pass.
                qstate = {}

                def q_pos(pair, g):
                    if g == 0:
                        qstate['ps'] = [
                            pbc.tile([P, SC], F32, tag="pout", bufs=2,
                                     name="q3_ps%d" % m) for m in pair]
                        qstate['xt'] = {}
                        for kk in (0, 1, 2, 3):
                            xq = qstate['xt'][kk] = xin.tile([P, SC], F16,
                                                              name="xq")
                            nc.sync.dma_start(
                                out=xq, in_=xT[kk * P:(kk + 1) * P, s3:s3 + SC])
                    for kk in (2 * g + 4, 2 * g + 5):
                        if kk < NKT:
                            xq = qstate['xt'][kk] = xin.tile([P, SC], F16,
                                                             name="xq")
                            nc.sync.dma_start(
                                out=xq, in_=xT[kk * P:(kk + 1) * P, s3:s3 + SC])
                    for kk in (2 * g, 2 * g + 1):
                        xq = qstate['xt'].pop(kk)
                        for i, m in enumerate(pair):
                            nc.tensor.matmul(qstate['ps'][i],
                                             lhsT=wq[:, kk, m * P:(m + 1) * P],
                                             rhs=xq, start=kk == 0,
                                             stop=kk == NKT - 1)
                    if g == NKT // 2 - 1:
                        for i, m in enumerate(pair):
                            nc.vector.tensor_copy(out=qT[:, m, s3:s3 + SC],
                                                  in_=qstate['ps'][i])

                for pair in ((0, 1), (2, 3)):
                    for g in range(NKT // 2):
                        work.append(("q", pair, g))

                def run_item(item, cp_eng=0):
                    if item[0] == "q":
                        q_pos(item[1], item[2])
                    else:
                        c_group(item[1], item[2], cp_eng=cp_eng)

                NG = NJT // 2  # score groups of 2 key tiles
                for t in range(NCH):
                    i0 = t * SC
                    for h in range(NH):
                        ctx_ps = pbc.tile([P, SC], F32, tag="pctx", bufs=2,
                                          name="ctx_ps")
                        racc = rpool.tile([P, SC], F16, name="racc", bufs=2)
                        e_tiles = {}
                        for g in range(NG + 2):
                            if g < NG:
                                s2 = pbc.tile([P, 2 * SC], F32, tag="pscore",
                                              bufs=2, name="s2")
                                j0 = 2 * g
                                nc.tensor.matmul(s2[:, 0:SC],
                                                 lhsT=kT[:, j0 * P:(j0 + 1) * P],
                                                 rhs=qT[:, h, i0:i0 + SC],
                                                 start=True, stop=True)
                                nc.tensor.matmul(s2[:, SC:2 * SC],
                                                 lhsT=kT[:, (j0 + 1) * P:(j0 + 2) * P],
                                                 rhs=qT[:, h, i0:i0 + SC],
                                                 start=True, stop=True)
                                e2 = epool.tile([P, 2 * SC], F16)
                                # exp(s*scale - 1): the -1 keeps fp16
                                # rowsums well inside range; it cancels in
                                # the softmax normalization.
                                nc.scalar.activation(out=e2, in_=s2,
                                                     func=EXP, scale=SCALE,
                                                     bias=nbias)
                                e_tiles[g] = e2
                            g2 = g - 2
                            if g2 >= 0:
                                e2 = e_tiles.pop(g2)
                                for half in range(2):
                                    j2 = 2 * g2 + half
                                    es = e2[:, half * SC:(half + 1) * SC]
                                    nc.tensor.matmul(ctx_ps, lhsT=v[:, j2, :],
                                                     rhs=es, start=j2 == 0,
                                                     stop=j2 == NJT - 1)
                                    if j2 == 0:
                                        nc.vector.tensor_copy(out=racc, in_=es)
                                    else:
                                        nc.vector.tensor_add(out=racc,
                                                             in0=racc, in1=es)
                                if work:
                                    run_item(work.pop(0))
                        # cross-partition rowsum broadcast via ones-matmul,
                        # then 1/r = exp(-ln(r)) on Act (no table switch)
                        rb_ps = pbc.tile([P, SC], F32, tag="pscore", bufs=2,
                                         name="rb_ps")
                        nc.tensor.matmul(rb_ps, lhsT=onesf, rhs=racc,
                                         start=True, stop=True)
                        lnr = rpool.tile([P, SC], F32, name="lnr", bufs=2)
                        nc.scalar.activation(out=lnr, in_=rb_ps, func=LN)
                        rbc = rpool.tile([P, SC], F32, name="rbc", bufs=2)
                        nc.scalar.activation(out=rbc, in_=lnr, func=EXP,
                                             scale=-1.0)
                        nc.vector.tensor_mul(out=ctxT[:, h, i0:i0 + SC],
                                             in0=ctx_ps, in1=rbc)
                    # enqueue this chunk's output-projection tiles; they run
                    # interleaved inside B(t+1) (or in the drain loop below)
                    for mt in range(t * NCH, (t + 1) * NCH):
                        for oc in range(NOC):
                            work.append(("c", mt, oc))
                drain_i = 0
                while work:
                    # alternate drain copies DVE/Act (Act is idle here)
                    run_item(work.pop(0), cp_eng=drain_i % 2)
                    drain_i += 1
    nc.finalize()
    return nc


def _get_program():
    if "nc" not in _CACHE:
        _CACHE["nc"] = _build()
    return _CACHE["nc"]


def _prep_inputs(hidden_states, Wq, Wk, Wv, bv, Wo):
    x = np.asarray(hidden_states, np.float32).reshape(S, HID)
    xT = np.ascontiguousarray(x.T).astype(F16NP)
    Wq = np.asarray(Wq, np.float32)
    Wk = np.asarray(Wk, np.float32)
    Wv = np.asarray(Wv, np.float32)
    bv = np.asarray(bv, np.float32)
    Wo = np.asarray(Wo, np.float32)
    maps = []
    for c in range(NCORES):
        qs = slice(c * DQ, (c + 1) * DQ)
        ks = slice(c * P, (c + 1) * P)
        maps.append({
            "xT": xT,
            "WqT": np.ascontiguousarray(Wq[qs].T).astype(F16NP),
            "WkT": np.ascontiguousarray(Wk[ks].T).astype(F16NP),
            "WvT": np.ascontiguousarray(Wv[ks].T).astype(F16NP),
            "bvp": np.ascontiguousarray(bv[ks]).reshape(P, 1),
            "WoT": np.ascontiguousarray(Wo[:, qs].T).astype(F16NP),
            "onesd": np.ones((P, P), F16NP),
        })
    return maps


def kernel(hidden_states, Wq, Wk, Wv, bv, Wo, _trace=False, **kw):
    nc = _get_program()
    maps = _prep_inputs(hidden_states, Wq, Wk, Wv, bv, Wo)
    res = run_bass_kernel_spmd(nc, maps, list(range(NCORES)), trace=_trace, **kw)
    out = np.zeros((S, HID), np.float32)
    for c in range(NCORES):
        out += np.asarray(res.results[c]["out"], np.float32)
    if _trace:
        return out.reshape(1, S, HID), res
    return out.reshape(1, S, HID)


# revision 20
# speedup vs baseline: 1.0897x; 1.0897x over previous
"""Grouped-Query Attention (S=2048, NQ=32, NKV=8, D=128, HID=4096) on 8 TRN2 NeuronCores.

Sharding: tensor-parallel over heads. Core c owns KV head c and its G=4
query heads (rows c*512..(c+1)*512 of Wq, c*128..(c+1)*128 of Wk/Wv, and
columns c*512..(c+1)*512 of Wo).  Each core computes a partial output
(row-parallel Wo); the host sums the 8 partials.

All matmuls run in fp16 (1 cycle/row on PE, better mantissa than bf16)
with fp32 PSUM accumulation.  Schedule:
  - stage A: projections, streaming x once; produces qT/kT/vT [d, s]
    (d on partitions) plus v[j, d] via 16 PE transposes.
  - stage B: scores computed transposed S^T[j,i] = kT-slice.T @ qT,
    exp on Act (scale folded in, bias -1 for fp16 range), rowsums
    accumulated on DVE in fp16 (2x mode), cross-partition sum via a
    ones-matmul, 1/r as exp(-ln(r)) on Act (ln+exp live in the same
    activation table, so no table reloads).
  - stage C: output projection, software-pipelined INTO stage B: one
    [128,512] output tile (4 accumulating matmuls) is issued after every
    odd ctx matmul, so the PE stays busy while Act drains the exps.
    C(t-1) runs inside B(t); per t the 32 output tiles exactly fill the
    4*8 interleave slots.
  - the 4MB wo DMA is issued after stage A's first chunk so it does not
    block the x stream / small weights in the single DMA queue (this was
    a 30us PE stall at startup).
"""

import os
import sys

import numpy as np

for _p in ("/opt/trn_rl_repo", "/root/.axon_site/_ro/trn_rl_repo"):
    if os.path.isdir(_p) and _p not in sys.path:
        sys.path.insert(0, _p)

import concourse.bass as bass
import concourse.bacc as bacc
import concourse.mybir as mybir
import concourse.tile as tile
from concourse.bass_utils import run_bass_kernel_spmd
from concourse.masks import make_identity

P = 128          # partitions / head dim / PE tile
S = 2048         # sequence length
HID = 4096       # hidden dim
NCORES = 8
NH = 4           # q heads per core
DQ = NH * P      # per-core q width (512)
SC = 512         # free-dim chunk (PSUM bank = 512 fp32)
NKT = HID // P   # 32 contraction tiles over hidden
NCH = S // SC    # 4 sequence chunks
NJT = S // P     # 16 key tiles
NOC = HID // SC  # 8 out column chunks
SCALE = float(P) ** -0.5
F16 = mybir.dt.float16
F32 = mybir.dt.float32
F16NP = np.float16

_CACHE = {}


def _build():
    nc = bacc.Bacc(None, target_bir_lowering=False)
    xT = nc.declare_dram_parameter("xT", [HID, S], F16, isOutput=False)
    # Weights arrive pre-gathered on the host into the exact SBUF layout,
    # so every DMA descriptor is a full contiguous partition row (8KB+)
    # instead of 256B gather lines (which run at half DMA rate).
    Wqp = nc.declare_dram_parameter("Wqp", [P, NKT, DQ], F16, isOutput=False)
    Wkp = nc.declare_dram_parameter("Wkp", [P, NKT, P], F16, isOutput=False)
    Wvp = nc.declare_dram_parameter("Wvp", [P, NKT, P], F16, isOutput=False)
    bvp = nc.declare_dram_parameter("bvp", [P, 1], F32, isOutput=False)
    Wop = nc.declare_dram_parameter("Wop", [P, NH, HID], F16, isOutput=False)
    onesd = nc.declare_dram_parameter("onesd", [P, P], F16, isOutput=False)
    out = nc.declare_dram_parameter("out", [S, HID], F16, isOutput=True)

    EXP = mybir.ActivationFunctionType.Exp
    LN = mybir.ActivationFunctionType.Ln
    IDENT = mybir.ActivationFunctionType.Identity

    with tile.TileContext(nc) as tc:
        with (
            tc.tile_pool(name="consts", bufs=1) as consts,
            tc.tile_pool(name="acts", bufs=1) as acts,
            tc.tile_pool(name="xin", bufs=16) as xin,
            tc.tile_pool(name="epool", bufs=5) as epool,
            tc.tile_pool(name="rpool", bufs=2) as rpool,
            tc.tile_pool(name="opool", bufs=8) as opool,
        ):
            # ---- constants first (warmup matmuls depend only on these).
            # Queue order matters: everything the first stage-A matmuls
            # need (wk, wv, wq) goes first; wo is DMA'd later.
            ident = consts.tile([P, P], F16)
            make_identity(nc, ident)
            # Preload the one activation table that serves Exp, Ln AND
            # Identity (act_info.json act_func_sets[6],
            # "natural_log_exp_and_others").  Without this the table-load
            # pass greedily ping-pongs between the exp-only and ln-only
            # tables: 33 ACT_TABLE_LOADs (~42us) on the Act critical path.
            _tl = mybir.InstLoadActFuncSet(
                name=nc.get_next_instruction_name(), ins=[], outs=[])
            _tl.act_func_set_id = 6
            nc.scalar.add_instruction(_tl)
            nbias = consts.tile([P, 1], F32)
            nc.gpsimd.memset(nbias, -1.0)
            # All weights go through the Act engine's DMA queue so the x
            # stream (sync queue) is never stuck behind them; the two HWDGE
            # queues drain in parallel.  Split wk/wv/wq into kt-octet parts
            # interleaved in deadline order, so the first k/v/q matmuls can
            # start after ~0.5MB instead of after 6MB.
            wk = consts.tile([P, NKT, P], F16)
            wv = consts.tile([P, NKT, P], F16)
            wq = consts.tile([P, NKT, DQ], F16)
            for g in range(4):
                sl = slice(g * 8, (g + 1) * 8)
                nc.scalar.dma_start(out=wk[:, sl, :], in_=Wkp[:, sl, :])
                nc.scalar.dma_start(out=wv[:, sl, :], in_=Wvp[:, sl, :])
                nc.scalar.dma_start(out=wq[:, sl, :], in_=Wqp[:, sl, :])
            bv_sb = consts.tile([P, 1], F32)
            nc.scalar.dma_start(out=bv_sb, in_=bvp[:, :])
            onesf = consts.tile([P, P], F16)
            nc.scalar.dma_start(out=onesf, in_=onesd[:, :])
            wo = consts.tile([P, NH, HID], F16)
            nc.scalar.dma_start(out=wo, in_=Wop[:, :, :])

            # ---- persistent activations (fp16) ----
            qT = acts.tile([P, NH, S], F16)     # per head: [128 d, 2048 s]
            kT = acts.tile([P, S], F16)         # [128 d, 2048 s]
            vT = acts.tile([P, S], F16)         # [128 d, 2048 s]
            v = acts.tile([P, NJT, P], F16)     # [128 j, jt, 128 d]
            ctxT = acts.tile([P, NH, S], F16)   # per head: [128 d, 2048 i]

            # ---- PE warmup: keep TensorE busy during initial weight DMAs so
            # the HAM clock-gate is released before real matmuls start ----
            with tc.tile_pool(name="pwarm", bufs=1, space="PSUM") as pwarm:
                wt = pwarm.tile([P, P], F16, name="warm")
                for _ in range(36):
                    nc.tensor.transpose(wt, ident, ident)

            # ---- stage A: projections (stream x once).  Chunk 0 lags the
            # q matmuls by LAG k-tiles so the k/v matmuls cover the wq DMA.
            # Chunk 3 skips q entirely: its q-projection is interleaved into
            # B(t=0) below, where it plays the role the output projection
            # plays in later B chunks (PE filler while Act drains exps). ----
            LAG = 4
            with tc.tile_pool(name="pacc", bufs=1, space="PSUM") as pacc:
                # Issue ALL x-tile DMAs up front (textual order = sync-queue
                # order).  The 16-buffer ring auto-throttles: tile i's DMA
                # fires once tile i-16's consumers finish, so each chunk's
                # tiles prefetch during the previous chunk's compute.  This
                # keeps the k/v-only stretches (chunk-0 lag phase, chunk 3)
                # from starving on DMA rate.
                all_x = {}
                for c in range(NCH):
                    for kt in range(NKT):
                        xt = all_x[(c, kt)] = xin.tile([P, SC], F16, name="xt")
                        nc.sync.dma_start(
                            out=xt,
                            in_=xT[kt * P:(kt + 1) * P, c * SC:(c + 1) * SC])
                for c in range(NCH):
                    s0 = c * SC
                    do_q = c != NCH - 1
                    lag = LAG if c == 0 else 0
                    if do_q:
                        q_ps = [pacc.tile([P, SC], F32, tag="pq%d" % m,
                                          name="q_ps%d" % m) for m in range(NH)]
                    k_ps = pacc.tile([P, SC], F32, tag="pk")
                    v_ps = pacc.tile([P, SC], F32, tag="pv")
                    for kt in range(NKT + lag):
                        if kt < NKT:
                            xt = all_x[(c, kt)]
                            st, sp = kt == 0, kt == NKT - 1
                            nc.tensor.matmul(k_ps, lhsT=wk[:, kt, :], rhs=xt,
                                             start=st, stop=sp)
                            nc.tensor.matmul(v_ps, lhsT=wv[:, kt, :], rhs=xt,
                                             start=st, stop=sp)
                        qk = kt - lag
                        if do_q and 0 <= qk < NKT:
                            xq = all_x[(c, qk)]
                            for m in range(NH):
                                nc.tensor.matmul(q_ps[m],
                                                 lhsT=wq[:, qk, m * P:(m + 1) * P],
                                                 rhs=xq, start=qk == 0,
                                                 stop=qk == NKT - 1)
                    # PSUM copy-out, split DVE/Act and ordered so the next
                    # chunk's matmuls (k, v, q0..q3) find their banks free
                    # as they need them.
                    nc.vector.tensor_copy(out=kT[:, s0:s0 + SC], in_=k_ps)
                    # v = x @ Wv.T + bv  (bias is per-partition in [d, s] layout)
                    nc.scalar.activation(out=vT[:, s0:s0 + SC], in_=v_ps,
                                         func=IDENT, bias=bv_sb, scale=1.0)
                    if do_q:
                        nc.vector.tensor_copy(out=qT[:, 0, s0:s0 + SC], in_=q_ps[0])
                        nc.vector.tensor_copy(out=qT[:, 1, s0:s0 + SC], in_=q_ps[1])
                        nc.scalar.activation(out=qT[:, 2, s0:s0 + SC], in_=q_ps[2],
                                             func=IDENT, scale=1.0)
                        nc.scalar.activation(out=qT[:, 3, s0:s0 + SC], in_=q_ps[3],
                                             func=IDENT, scale=1.0)
                    # v[j, d] via PE transpose, interleaved per chunk
                    for jj in range(SC // P):
                        jt = c * (SC // P) + jj
                        t_ps = pacc.tile([P, P], F16, tag="ptr", bufs=2)
                        nc.tensor.transpose(t_ps, vT[:, jt * P:(jt + 1) * P], ident)
                        nc.vector.tensor_copy(out=v[:, jt, :], in_=t_ps)

            # ---- stages B+C: attention with PE filler work software-
            # pipelined into the key-tile loop.  Per (t,h) slot the jt loop
            # yields 8 interleave positions; fillers are closures popped
            # from `work`: at t=0 the deferred chunk-3 q-projection (32
            # positions), at t>0 the previous chunk's output projection
            # (32 tiles).  PSUM: pscore 2x[128,1024] + pctx 2 + pout 2 =
            # exactly 8 banks. ----
            s3 = (NCH - 1) * SC
            with tc.tile_pool(name="pbc", bufs=1, space="PSUM") as pbc:
                work = []

                def c_group(mt, oc, cp_eng=0):
                    m0, o0 = mt * P, oc * SC
                    o_ps = pbc.tile([P, SC], F32, tag="pout", bufs=2, name="o_ps")
                    for dt_ in range(NH):
                        nc.tensor.matmul(o_ps, lhsT=ctxT[:, dt_, m0:m0 + P],
                                         rhs=wo[:, dt_, o0:o0 + SC],
                                         start=dt_ == 0, stop=dt_ == NH - 1)
                    ob = opool.tile([P, SC], F16)
                    if cp_eng == 0:
                        nc.vector.tensor_copy(out=ob, in_=o_ps)
                    else:
                        nc.scalar.activation(out=ob, in_=o_ps, func=IDENT,
                                             scale=1.0)
                    nc.sync.dma_start(out=out[m0:m0 + P, o0:o0 + SC], in_=ob)

                # Deferred chunk-3 q-projection: two passes (head pairs
                # (0,1) then (2,3)), each re-streaming x chunk 3; position g
                # covers k-tiles 2g,2g+1 for both heads of the pass.
                qstate = {}

                def q_pos(pair, g):
                    if g == 0:
                        qstate['ps'] = [
                            pbc.tile([P, SC], F32, tag="pout", bufs=2,
                                     name="q3_ps%d" % m) for m in pair]
                        qstate['xt'] = {}
                        for kk in (0, 1, 2, 3):
                            xq = qstate['xt'][kk] = xin.tile([P, SC], F16,
                                                              name="xq")
                            nc.sync.dma_start(
                                out=xq, in_=xT[kk * P:(kk + 1) * P, s3:s3 + SC])
                    for kk in (2 * g + 4, 2 * g + 5):
                        if kk < NKT:
                            xq = qstate['xt'][kk] = xin.tile([P, SC], F16,
                                                             name="xq")
                            nc.sync.dma_start(
                                out=xq, in_=xT[kk * P:(kk + 1) * P, s3:s3 + SC])
                    for kk in (2 * g, 2 * g + 1):
                        xq = qstate['xt'].pop(kk)
                        for i, m in enumerate(pair):
                            nc.tensor.matmul(qstate['ps'][i],
                                             lhsT=wq[:, kk, m * P:(m + 1) * P],
                                             rhs=xq, start=kk == 0,
                                             stop=kk == NKT - 1)
                    if g == NKT // 2 - 1:
                        for i, m in enumerate(pair):
                            nc.vector.tensor_copy(out=qT[:, m, s3:s3 + SC],
                                                  in_=qstate['ps'][i])

                for pair in ((0, 1), (2, 3)):
                    for g in range(NKT // 2):
                        work.append(("q", pair, g))

                def run_item(item, cp_eng=0):
                    if item[0] == "q":
                        q_pos(item[1], item[2])
                    else:
                        c_group(item[1], item[2], cp_eng=cp_eng)

                NG = NJT // 2  # score groups of 2 key tiles
                for t in range(NCH):
                    i0 = t * SC
                    for h in range(NH):
                        ctx_ps = pbc.tile([P, SC], F32, tag="pctx", bufs=2,
                                          name="ctx_ps")
                        racc = rpool.tile([P, SC], F16, name="racc", bufs=2)
                        e_tiles = {}
                        for g in range(NG + 2):
                            if g < NG:
                                s2 = pbc.tile([P, 2 * SC], F32, tag="pscore",
                                              bufs=2, name="s2")
                                j0 = 2 * g
                                nc.tensor.matmul(s2[:, 0:SC],
                                                 lhsT=kT[:, j0 * P:(j0 + 1) * P],
                                                 rhs=qT[:, h, i0:i0 + SC],
                                                 start=True, stop=True)
                                nc.tensor.matmul(s2[:, SC:2 * SC],
                                                 lhsT=kT[:, (j0 + 1) * P:(j0 + 2) * P],
                                                 rhs=qT[:, h, i0:i0 + SC],
                                                 start=True, stop=True)
                                e2 = epool.tile([P, 2 * SC], F16)
                                # exp(s*scale - 1): the -1 keeps fp16
                                # rowsums well inside range; it cancels in
                                # the softmax normalization.
                                nc.scalar.activation(out=e2, in_=s2,
                                                     func=EXP, scale=SCALE,
                                                     bias=nbias)
                                e_tiles[g] = e2
                            g2 = g - 2
                            if g2 >= 0:
                                e2 = e_tiles.pop(g2)
                                for half in range(2):
                                    j2 = 2 * g2 + half
                                    es = e2[:, half * SC:(half + 1) * SC]
                                    nc.tensor.matmul(ctx_ps, lhsT=v[:, j2, :],
                                                     rhs=es, start=j2 == 0,
                                                     stop=j2 == NJT - 1)
                                    if j2 == 0:
                                        nc.vector.tensor_copy(out=racc, in_=es)
                                    else:
                                        nc.vector.tensor_add(out=racc,
                                                             in0=racc, in1=es)
                                if work:
                                    run_item(work.pop(0))
                        # cross-partition rowsum broadcast via ones-matmul,
                        # then 1/r = exp(-ln(r)) on Act (no table switch)
                        rb_ps = pbc.tile([P, SC], F32, tag="pscore", bufs=2,
                                         name="rb_ps")
                        nc.tensor.matmul(rb_ps, lhsT=onesf, rhs=racc,
                                         start=True, stop=True)
                        lnr = rpool.tile([P, SC], F32, name="lnr", bufs=2)
                        nc.scalar.activation(out=lnr, in_=rb_ps, func=LN)
                        rbc = rpool.tile([P, SC], F32, name="rbc", bufs=2)
                        nc.scalar.activation(out=rbc, in_=lnr, func=EXP,
                                             scale=-1.0)
                        nc.vector.tensor_mul(out=ctxT[:, h, i0:i0 + SC],
                                             in0=ctx_ps, in1=rbc)
                    # enqueue this chunk's output-projection tiles; they run
                    # interleaved inside B(t+1) (or in the drain loop below)
                    for mt in range(t * NCH, (t + 1) * NCH):
                        for oc in range(NOC):
                            work.append(("c", mt, oc))
                drain_i = 0
                while work:
                    # alternate drain copies DVE/Act (Act is idle here)
                    run_item(work.pop(0), cp_eng=drain_i % 2)
                    drain_i += 1
    nc.finalize()
    return nc


def _get_program():
    if "nc" not in _CACHE:
        _CACHE["nc"] = _build()
    return _CACHE["nc"]


def _prep_inputs(hidden_states, Wq, Wk, Wv, bv, Wo):
    x = np.asarray(hidden_states, np.float32).reshape(S, HID)
    xT = np.ascontiguousarray(x.T).astype(F16NP)
    Wq = np.asarray(Wq, np.float32)
    Wk = np.asarray(Wk, np.float32)
    Wv = np.asarray(Wv, np.float32)
    bv = np.asarray(bv, np.float32)
    Wo = np.asarray(Wo, np.float32)
    maps = []
    for c in range(NCORES):
        qs = slice(c * DQ, (c + 1) * DQ)
        ks = slice(c * P, (c + 1) * P)
        # Pre-gather into the SBUF layouts: tile[p, kt, col] = W.T[kt*P+p, col]
        wqp = Wq[qs].reshape(DQ, NKT, P).transpose(2, 1, 0)
        wkp = Wk[ks].reshape(P, NKT, P).transpose(2, 1, 0)
        wvp = Wv[ks].reshape(P, NKT, P).transpose(2, 1, 0)
        wop = Wo[:, qs].T.reshape(NH, P, HID).transpose(1, 0, 2)
        maps.append({
            "xT": xT,
            "Wqp": np.ascontiguousarray(wqp).astype(F16NP),
            "Wkp": np.ascontiguousarray(wkp).astype(F16NP),
            "Wvp": np.ascontiguousarray(wvp).astype(F16NP),
            "bvp": np.ascontiguousarray(bv[ks]).reshape(P, 1),
            "Wop": np.ascontiguousarray(wop).astype(F16NP),
            "onesd": np.ones((P, P), F16NP),
        })
    return maps


def kernel(hidden_states, Wq, Wk, Wv, bv, Wo, _trace=False, **kw):
    nc = _get_program()
    maps = _prep_inputs(hidden_states, Wq, Wk, Wv, bv, Wo)
    res = run_bass_kernel_spmd(nc, maps, list(range(NCORES)), trace=_trace, **kw)
    out = np.zeros((S, HID), np.float32)
    for c in range(NCORES):
        out += np.asarray(res.results[c]["out"], np.float32)
    if _trace:
        return out.reshape(1, S, HID), res
    return out.reshape(1, S, HID)
